# revision 19
# baseline (speedup 1.0000x reference)
"""Trainium2 Bass kernel for an AttentionBlock (b=8, c=512, T=32*64=2048, 4 heads).

Data-parallel over batch: each of the 8 NeuronCores processes one batch
element end-to-end (QKV projection, attention, output projection,
residual).  Weights are replicated; no collectives.

Dtype plan (gate is rel_max < 2e-2; this lands 8.4e-3 on HW):
  - QKV / scores / PV / denominator / output projection are all fp8e4m3
    DoubleRow matmuls (0.5 cycles/row).
  - exp outputs are fp8e4m3 of exp(s - 5): softmax is invariant to a
    constant score shift, and max score on this data is ~6.3, so
    exp(s-5) <= ~4 stays far from e4m3's 448 ceiling.  The shift is
    applied INSIDE the score matmul: the DoubleRow second plane (which
    would otherwise contract zeros) holds q-plane 1.0 x k-plane
    -5/128, adding exactly -5 to every score for free - an ACT bias
    operand would cost ~200ns per exp op on HW.
  - The denominator D[t] = sum_s e[s,t] is summed on the PE from the
    SAME quantized e tiles the PV matmuls consume (ones-lhsT DoubleRow
    colsum matmuls), so the softmax weights stay exactly consistent.
  - V^T is also fp8 (enables the fp8 DR PV matmul).
  - K bias is dropped entirely: it adds a per-t constant to every score
    row, which cancels in softmax over s.  v-bias is folded into the
    projection bias on the host (exact, since softmax rows sum to 1).
  - 1/D: nc.vector.reciprocal_approx_fast (single custom-DVE op, ~51
    ULP) reading the PSUM D row directly; the [128,512] broadcast of
    1/D runs on the otherwise-idle GpSimd (partition_broadcast), not as
    a PE ones-matmul.

The kernel is PE-SEQUENCER-bound, not engine-throughput-bound: every
matmul legalizes to Ldweights+Matmult and carries a counting-semaphore
update, ~145ns/instruction of sequencer time.  The design therefore
minimizes PE instruction count (640 matmuls: 256 score + 128 PV + 128
denominator + 128 QKV/proj tiles) and keeps every other engine's work
strictly smaller: ACT runs one [128,2x512] exp per score group (~0.7us,
2x fp8 write mode), DVE does the PSUM->SBUF stores, reciprocal and
normalize-mul, GpSimd does broadcasts and output DMAs.

Scheduling: the PE FIFO is strict in-order, so emission order is
execution order.  Q/V^T projection tiles trickle in as 1-per-step PE
fillers inside the score+exp stream; PV lags the exp stream by PVLAG
e-pairs; the per-iteration denominator burst fires 2 steps after the
iteration's last PV (its newest e tile is then old enough that the PE
FIFO never head-of-line-blocks on the exp semaphore) and the normalize
chain is staggered over +4..+6 steps; projection chunks follow one per
step.  The benchmark loop uses For_i(staggered_reset=True) so
consecutive iterations overlap their input DMAs with the previous
iteration's tail.
"""

import math

import numpy as np

import concourse.bacc as bacc
import concourse.mybir as mybir
import concourse.tile as tile
from concourse.bass_utils import run_bass_kernel_spmd

P = 128          # partitions
C = 512          # channels
T = 2048         # tokens (f*t = 32*64)
H = 4            # heads (head dim = 128 = P)
B = 8            # batch (one per core)
NC_ = C // P     # 4 c-chunks
NT = T // 512    # 4 t-blocks
NS = T // P      # 16 s-tiles
FP = mybir.dt.float32
BF = mybir.dt.bfloat16
F8 = mybir.dt.float8e4
DR = mybir.MatmulPerfMode.DoubleRow
EXP_GRP = 2      # s-tiles per exp() call ([128, 1024] PSUM group)
SHIFT = 5.0      # constant score shift inside exp (cancels in softmax)

_PROGRAM_CACHE = {}


def _build_program(loop_k: int = 1, probe=None):
    nc = bacc.Bacc()

    x8_d = nc.dram_tensor("x8", [C, T], F8, kind="ExternalInput")
    xbf_d = nc.dram_tensor("xbf", [C, T], BF, kind="ExternalInput")
    wq8_d = nc.dram_tensor("wq8", [C, C], F8, kind="ExternalInput")  # [c,o], scale folded
    wk8_d = nc.dram_tensor("wk8", [C, C], F8, kind="ExternalInput")
    wv8_d = nc.dram_tensor("wv8", [C, C], F8, kind="ExternalInput")
    pw8_d = nc.dram_tensor("pw8", [C, C], F8, kind="ExternalInput")
    bq_d = nc.dram_tensor("bq", [P, NC_], FP, kind="ExternalInput")  # [p, chunk]
    pb_d = nc.dram_tensor("pb", [P, NC_], FP, kind="ExternalInput")  # pb + Pw@bv
    zq_d = nc.dram_tensor("zq8", [P, T], F8, kind="ExternalInput")   # 1.0
    zk_d = nc.dram_tensor("zk8", [P, T], F8, kind="ExternalInput")   # -SHIFT/128
    out_d = nc.dram_tensor("out", [C, T], BF, kind="ExternalOutput")

    x8_v = x8_d.rearrange("(j p) t -> p j t", p=P)    # [128, 4, 2048]
    xbf_v = xbf_d.rearrange("(j p) t -> p j t", p=P)
    wq_v = wq8_d.rearrange("(j p) o -> p j o", p=P)
    wk_v = wk8_d.rearrange("(j p) o -> p j o", p=P)
    wv_v = wv8_d.rearrange("(j p) o -> p j o", p=P)
    pw_v = pw8_d.rearrange("(j p) o -> p j o", p=P)
    out_v = out_d.rearrange("(j p) t -> p j t", p=P)

    with tile.TileContext(nc) as tc:
        with tc.tile_pool(name="outer", bufs=1) as outer:
            # Constants that survive across loop iterations: the ones
            # vectors and the shared shift planes (index H) inside q8/k8
            # (written once, read-only in the loop body).  The score
            # matmuls address head h's data + the shift plane as a strided
            # 2-plane AP; q-plane 1.0 x k-plane -SHIFT/128 adds -SHIFT to
            # every score during the matmul (exact in fp8/fp32), so the
            # exp needs no bias operand (a bias AP costs ~200ns/op on HW).
            q8_sb = outer.tile([P, H + 1, T], F8, name="q8")  # slot H == 1.0
            k8_sb = outer.tile([P, H + 1, T], F8, name="k8")  # slot H == -SHIFT/128
            ones_row = outer.tile([1, P], BF)      # lhsT for bcast matmul
            # lhsT for the D colsum (DR): DoubleRow LdWeights requires the
            # 2-plane step to be a multiple of 16 bytes, so pad to [P,2,16]
            # and slice [:, :, 0:1]
            ones2_t = outer.tile([P, 2, 16], F8)
            ones_row_f = outer.tile([1, P], FP)
            ones2_f = outer.tile([P, 2, 16], FP)
            nc.vector.memset(ones_row_f, 1.0)
            nc.vector.memset(ones2_f, 1.0)
            nc.vector.tensor_copy(ones_row, ones_row_f)
            nc.vector.tensor_copy(ones2_t, ones2_f)
            ones2 = ones2_t[:, :, 0:1]
            # shift planes come from DRAM via the GpSimd DGE (a DVE
            # memset or an SP-queue DMA would block the K-projection path)
            nc.gpsimd.dma_start(q8_sb[:, H, :], zq_d[:])
            nc.gpsimd.dma_start(k8_sb[:, H, :], zk_d[:])

            dram_views = (x8_v, xbf_v, wq_v, wk_v, wv_v, pw_v,
                          bq_d, pb_d, out_v)
            consts = (ones_row, ones2)
            with (
                tc.tile_pool(name="pp", bufs=1) as pp,
                tc.tile_pool(name="psA", bufs=3, space="PSUM") as psA,
                tc.tile_pool(name="psAcc", bufs=2, space="PSUM") as psAcc,
                tc.tile_pool(name="epool", bufs=26) as epool,
                tc.tile_pool(name="anorm", bufs=2) as anormp,
                tc.tile_pool(name="small", bufs=2) as small,
            ):
                pools = (psA, psAcc, epool, anormp, small)
                tiles = _alloc_body_tiles(pp, 0)
                if loop_k > 1:
                    with tc.For_i(0, loop_k, staggered_reset=True):
                        _emit_kernel_body(
                            nc, tc, q8_sb, k8_sb, consts,
                            *dram_views, probe=probe, tiles=tiles, uname=0,
                            pools=pools,
                        )
                else:
                    _emit_kernel_body(
                        nc, tc, q8_sb, k8_sb, consts,
                        *dram_views, probe=probe, tiles=tiles, uname=0,
                        pools=pools,
                    )

    nc.compile()
    return nc


def _alloc_body_tiles(pool, u):
    return {
        "x8": pool.tile([P, NC_, T], F8, name=f"x8_{u}"),
        "xbf": pool.tile([P, NC_, T], BF, name=f"xbf_{u}"),
        "vt": pool.tile([P, NS, C], F8, name=f"vt_{u}"),
        "pw8": pool.tile([P, NC_, C], F8, name=f"pw8_{u}"),
        "bq": pool.tile([P, NC_], FP, name=f"bq_{u}"),
        "pb": pool.tile([P, NC_], FP, name=f"pb_{u}"),
        "wq": pool.tile([P, NC_, C], F8, name=f"wq_{u}"),
        "wk": pool.tile([P, NC_, C], F8, name=f"wk_{u}"),
        "wv": pool.tile([P, NC_, C], F8, name=f"wv_{u}"),
    }


def _emit_kernel_body(nc, tc, q8_sb, k8_sb, consts,
                      x8_v, xbf_v, wq_v, wk_v, wv_v, pw_v,
                      bq_d, pb_d, out_v, probe=None,
                      tiles=None, uname=0, pools=None):
    ones_row, ones2 = consts
    AF = mybir.ActivationFunctionType
    # probe modes (timing-only HW experiments; outputs are garbage):
    #   'se'     : QKV + scores+exp stream (no PV/denominator/normalize/proj)
    #   'nonorm' : full kernel minus normalize/proj/out-DMA
    p_qkv = probe in (None, 'se', 'nonorm')
    p_pv = probe in (None, 'nonorm')
    p_norm = probe is None

    psA, psAcc, epool, anormp, small = pools
    x8_sb = tiles["x8"]
    xbf_sb = tiles["xbf"]
    vt_sb = tiles["vt"]
    pw8_sb = tiles["pw8"]
    bq_sb = tiles["bq"]
    pb_sb = tiles["pb"]
    wq_sb = tiles["wq"]
    wk_sb = tiles["wk"]
    wv_sb = tiles["wv"]

    if p_qkv:
        nc.sync.dma_start(bq_sb, bq_d[:])

        # ---- phase A DMAs: K's dependencies first so exp starts early.
        # Few, large transfers: each dma_start costs ~0.7us of SP
        # sequencing, so per-chunk triggers would gate the pipeline start.
        nc.sync.dma_start(wk_sb, wk_v)
        nc.sync.dma_start(x8_sb[:, :, 0:512], x8_v[:, :, 0:512])
        nc.sync.dma_start(x8_sb[:, :, 512:1024], x8_v[:, :, 512:1024])
        nc.sync.dma_start(wq_sb, wq_v)
        nc.sync.dma_start(x8_sb[:, :, 1024:1536], x8_v[:, :, 1024:1536])
        nc.sync.dma_start(x8_sb[:, :, 1536:2048], x8_v[:, :, 1536:2048])
        nc.sync.dma_start(wv_sb, wv_v)
        nc.sync.dma_start(pw8_sb, pw_v)
        nc.sync.dma_start(pb_sb, pb_d[:])
        nc.sync.dma_start(xbf_sb[:, :, 0:1024], xbf_v[:, :, 0:1024])
        nc.sync.dma_start(xbf_sb[:, :, 1024:2048], xbf_v[:, :, 1024:2048])

    # Q/K projection tile: out[o_tile, t] = sum_g W8[c_g, ot].T @ x8[c_g, t]
    # (DoubleRow: two 128-channel k-tiles per matmul).  b_sb=None -> plain
    # copy (K has no bias: it cancels in softmax).
    def emit_qk_tile(w_sb, b_sb, dst, tb, ot, copy_eng="vector", pool=None):
        ps = psA.tile([P, 2, 512], FP, tag="mm",
                      name=f"kq{tb}_{ot}_{uname}")[:, 0, :]
        for g in range(2):
            nc.tensor.matmul(
                ps,
                w_sb[:, 2 * g:2 * g + 2, ot * P:(ot + 1) * P],
                x8_sb[:, 2 * g:2 * g + 2, tb * 512:(tb + 1) * 512],
                start=(g == 0),
                stop=(g == 1),
                perf_mode=DR,
            )
        dst_ap = dst[:, ot, tb * 512:(tb + 1) * 512]
        # bias-add/copy + fp8 store (ScalarE helps only where it would
        # otherwise idle, i.e. the startup K projection)
        if copy_eng == "act":
            if b_sb is None:
                nc.scalar.activation(dst_ap, ps, AF.Identity)
            else:
                nc.scalar.activation(dst_ap, ps, AF.Identity,
                                     bias=b_sb[:, ot:ot + 1])
        elif b_sb is None:
            nc.vector.tensor_copy(dst_ap, ps)
        else:
            nc.vector.tensor_scalar_add(dst_ap, ps, b_sb[:, ot:ot + 1])

    # V^T tile: out[s_tile, o] = sum_g x8[c_g, s_tile].T @ Wv8[c_g, o]
    def emit_vt_tile(st, pool=None):
        ps = psA.tile([P, 2, 512], FP, tag="mm",
                      name=f"vt{st}_{uname}")[:, 0, :]
        for g in range(2):
            nc.tensor.matmul(
                ps,
                x8_sb[:, 2 * g:2 * g + 2, st * P:(st + 1) * P],
                wv_sb[:, 2 * g:2 * g + 2, :],
                start=(g == 0),
                stop=(g == 1),
                perf_mode=DR,
            )
        nc.vector.tensor_copy(vt_sb[:, st, :], ps)

    # K first (scores for any t-block read all of k8), then the first Q
    # tile; the remaining Q/V^T tiles are interleaved into the score
    # stream as PE fillers.
    fillers = []
    if p_qkv:
        # alternate psA/psS so four K tiles are in flight (two copy
        # chains, ACT + DVE, truly overlapped) during the startup window
        for i, (tb, ot) in enumerate([(tb, ot) for tb in range(NT)
                                      for ot in range(NC_)]):
            emit_qk_tile(wk_sb, None, k8_sb, tb, ot,
                         copy_eng="act" if i % 2 else "vector",
                         pool=psA if i % 2 else None)
        emit_qk_tile(wq_sb, bq_sb, q8_sb, 0, 0, pool=psA)

        for ot in range(1, NC_):
            fillers.append(("q", 0, ot))
        for st in range(NS):
            fillers.append(("vt", st))
        for tb in range(1, NT):
            for ot in range(NC_):
                fillers.append(("q", tb, ot))

    def pop_fillers(step):
        if not fillers:
            return
        f = fillers.pop(0)
        if f[0] == "q":
            emit_qk_tile(wq_sb, bq_sb, q8_sb, f[1], f[2])
        else:
            emit_vt_tile(f[1])

    # ---- phase B/C: attention + projection, software-pipelined ----
    # The PE engine queue is strict FIFO, so emission order == PE
    # execution order.  One fine step = one s-tile: a single DR score
    # matmul [128,512] into a 1-bank psA tile (4-deep rotation, so up to
    # 4 MM->exp handoffs are in flight and semaphore latency amortizes),
    # then one flat [128,512] exp (1-D APs; a 2-D free AP or a bias
    # operand each cost extra ACT cycles on HW).  PV lags the exp stream
    # by PVLAG e-pairs; the per-iteration denominator burst fires after
    # the iteration's last PV and the normalize chain is staggered.
    PVLAG = 12          # e-pairs, while PE fillers pending
    PVLAG_STEADY = 4    # e-pairs, once fillers drained

    NGR = NS // EXP_GRP                    # 8 e-pairs per (h, tb)
    iters = [(h, tb) for tb in range(NT) for h in range(H)]
    NIT = len(iters)

    acc = {}      # it -> dict(a=, d=, dsb=, b=, r=)
    etiles = {}   # it -> list of the it's e tiles (for the D burst)
    an = {}       # tb -> an_sb tile

    def emit_pv(it, g, e_sb):
        h, tb = iters[it]
        if g == 0:
            acc[it] = {"a": psAcc.tile([P, 512], FP, tag="acc",
                                       name=f"aps{it}_{uname}")}
        nc.tensor.matmul(
            acc[it]["a"],
            vt_sb[:, 2 * g:2 * g + 2, h * P:(h + 1) * P],
            e_sb[:, :, :],
            start=(g == 0),
            stop=(g == NGR - 1),
            perf_mode=DR,
        )

    # --- staggered post-PV chain: each stage's cross-engine dependency
    # is several steps old when the consuming engine reaches it, so
    # exposed semaphore waits do not head-of-line-block the PE/DVE FIFOs. ---
    def ev_dsum(it):
        # denominator: D[t] = sum_s e[s,t], summed on PE from the SAME
        # fp8 e tiles the PV matmuls consume (softmax stays consistent)
        d_ps = psA.tile([P, 2, 512], FP, tag="mm",
                        name=f"dps{it}_{uname}")[0:1, 0, :]
        acc[it]["d"] = d_ps
        es = etiles.pop(it)
        for g, e_sb in enumerate(es):
            nc.tensor.matmul(d_ps, ones2, e_sb[:, :, :],
                             start=(g == 0), stop=(g == len(es) - 1),
                             perf_mode=DR)

    def ev_recip(it):
        # custom-DVE op reads the PSUM D row directly (saves a dcopy)
        r1 = small.tile([1, 512], FP, tag="r1", name=f"r1{it}_{uname}")
        acc[it]["r1"] = r1
        nc.vector.reciprocal_approx_fast(r1, acc[it]["d"])

    def ev_bcast(it):
        # 1/D broadcast to all partitions on the otherwise-idle GpSimd
        # (saves a ones-row matmul + ldweights on the PE sequencer)
        r_sb = small.tile([P, 512], FP, tag="rsb", name=f"rsb{it}_{uname}")
        acc[it]["r"] = r_sb
        nc.gpsimd.partition_broadcast(r_sb, acc[it]["r1"])

    def ev_mul(it):
        h, tb = iters[it]
        if h == 0:
            an[tb] = anormp.tile([P, NC_, 512], F8, tag="anorm",
                                 name=f"an{tb}_{uname}")
        nc.vector.tensor_mul(an[tb][:, h, :], acc[it]["a"], acc[it]["r"])
        acc.pop(it)

    def ev_proj(pl):
        tb, ot = pl
        tsl = slice(tb * 512, (tb + 1) * 512)
        an_sb = an[tb]
        hp = psA.tile([P, 2, 512], FP, tag="mm",
                      name=f"hp{tb}_{ot}_{uname}")[:, 0, :]
        for g in range(2):
            nc.tensor.matmul(
                hp,
                pw8_sb[:, 2 * g:2 * g + 2, ot * P:(ot + 1) * P],
                an_sb[:, 2 * g:2 * g + 2, :],
                start=(g == 0),
                stop=(g == 1),
                perf_mode=DR,
            )
        o_sb = small.tile([P, 512], BF, tag="osb", bufs=3)
        # out = (hp + pb') + x  in one DVE op
        nc.vector.scalar_tensor_tensor(
            o_sb, hp, pb_sb[:, ot:ot + 1], xbf_sb[:, ot, tsl],
            op0=mybir.AluOpType.add, op1=mybir.AluOpType.add,
        )
        nc.gpsimd.dma_start(out_v[:, ot, tsl], o_sb)

    EV = {"dsum": ev_dsum, "bcast": ev_bcast,
          "recip": ev_recip, "mul": ev_mul, "proj": ev_proj}
    events = []   # sorted (due_step, seq, kind, payload)
    ev_seq = [0]

    def push(due, kind, payload):
        import bisect
        item = (due, ev_seq[0], kind, payload)
        ev_seq[0] += 1
        bisect.insort(events, item)

    def flush(step):
        while events and events[0][0] <= step:
            _, _, kind, pl = events.pop(0)
            EV[kind](pl)

    def pop_pv(step):
        pit, pg, pe_sb = pv_q.pop(0)
        emit_pv(pit, pg, pe_sb)
        if pg == NGR - 1 and p_norm:
            # the D burst re-reads the it's LAST e tile, which the exp
            # stream only just produced - delay it so the PE FIFO never
            # head-of-line-blocks on the exp semaphore
            push(step + 2, "dsum", pit)
            push(step + 4, "recip", pit)
            push(step + 5, "bcast", pit)
            push(step + 6, "mul", pit)
            nh, ntb = iters[pit]
            if nh == H - 1:
                for k in range(NC_):
                    push(step + 7 + k, "proj", (ntb, k))
        elif pg == NGR - 1:
            etiles.pop(pit, None)
            acc.pop(pit)

    flat = [(it, g) for it in range(NIT) for g in range(NGR)]
    pv_q = []             # queue of (it, g, e_sb)
    for step, (it, g) in enumerate(flat):
        h, tb = iters[it]
        tsl = slice(tb * 512, (tb + 1) * 512)
        s_ps = psA.tile([P, EXP_GRP, 512], FP, tag="mm",
                        name=f"sps{it}_{g}_{uname}")
        for u in range(EXP_GRP):
            st = g * EXP_GRP + u
            nc.tensor.matmul(
                s_ps[:, u, :],
                k8_sb[:, h:H + 1:(H - h), st * P:(st + 1) * P],
                q8_sb[:, h:H + 1:(H - h), tsl],
                start=True,
                stop=True,
                perf_mode=DR,
            )
        e_sb = epool.tile([P, EXP_GRP, 512], F8, tag="e",
                          name=f"e{it}_{g}_{uname}")
        nc.scalar.activation(e_sb[:, :, :], s_ps[:, :, :], AF.Exp)
        pop_fillers(step)
        if p_pv:
            etiles.setdefault(it, []).append(e_sb)
            pv_q.append((it, g, e_sb))
            lag = PVLAG if fillers else PVLAG_STEADY
            while len(pv_q) > lag:
                pop_pv(step)
        flush(step)

    # drain the pipeline tail, one virtual step at a time
    step = len(flat)
    while pv_q or events:
        if pv_q:
            pop_pv(step)
        flush(step)
        step += 1


def _prepare_in_maps(x, qkv_w, qkv_b, proj_w, proj_b):
    import ml_dtypes

    scale = 1.0 / math.sqrt(math.sqrt(C // H))
    x = np.ascontiguousarray(np.asarray(x, dtype=np.float32).reshape(B, C, T))
    qkv_w = np.asarray(qkv_w, dtype=np.float32)
    qkv_b = np.asarray(qkv_b, dtype=np.float32)
    proj_w = np.asarray(proj_w, dtype=np.float32)
    proj_b = np.asarray(proj_b, dtype=np.float32)

    e4 = ml_dtypes.float8_e4m3
    bf = ml_dtypes.bfloat16
    wq8 = np.ascontiguousarray((qkv_w[0:C] * scale).T.astype(e4))      # [c, o]
    wk8 = np.ascontiguousarray((qkv_w[C:2 * C] * scale).T.astype(e4))
    wv8 = np.ascontiguousarray(qkv_w[2 * C:3 * C].T.astype(e4))
    pw8 = np.ascontiguousarray(proj_w.T.astype(e4))
    bq = np.ascontiguousarray((qkv_b[0:C] * scale).reshape(NC_, P).T)  # [p, chunk]
    # v-bias folded through the projection (exact: softmax rows sum to 1);
    # k-bias dropped (adds a per-t constant to scores -> cancels in softmax)
    pb2 = proj_w @ qkv_b[2 * C:3 * C] + proj_b
    pb = np.ascontiguousarray(pb2.reshape(NC_, P).T)

    shared = {
        "wq8": wq8, "wk8": wk8, "wv8": wv8, "pw8": pw8,
        "bq": bq, "pb": pb,
        # score-shift planes: q-plane 1.0 x k-plane -SHIFT/P sums to -SHIFT
        # across the 128 partitions of the DR second plane (exact in fp8)
        "zq8": np.ones((P, T), dtype=e4),
        "zk8": np.full((P, T), -SHIFT / P, dtype=e4),
    }
    return [
        {
            "x8": np.ascontiguousarray(x[i].astype(e4)),
            "xbf": np.ascontiguousarray(x[i].astype(bf)),
            **shared,
        }
        for i in range(B)
    ]


def run(inputs, trace=False, **spmd_kwargs):
    """Run the kernel; returns (output [8,512,32,64], BassKernelResults)."""
    if "nc" not in _PROGRAM_CACHE:
        _PROGRAM_CACHE["nc"] = _build_program()
    nc = _PROGRAM_CACHE["nc"]
    in_maps = _prepare_in_maps(
        inputs["x"], inputs["qkv_w"], inputs["qkv_b"],
        inputs["proj_w"], inputs["proj_b"],
    )
    res = run_bass_kernel_spmd(nc, in_maps, list(range(B)), trace=trace, **spmd_kwargs)
    out = np.stack(
        [np.asarray(res.results[i]["out"]).astype(np.float32) for i in range(B)]
    )
    f = 32
    return out.reshape(B, C, f, T // f), res


def kernel(x, qkv_w, qkv_b, proj_w, proj_b):
    out, _ = run(
        {"x": x, "qkv_w": qkv_w, "qkv_b": qkv_b, "proj_w": proj_w, "proj_b": proj_b}
    )
    return out
